# revision 15
# baseline (speedup 1.0000x reference)
"""ExplaiNN (dense_cnn) Trainium2 Bass kernel, 8-core SPMD. v3.

Pipeline per reference:
  conv1d(4->300 units, K=19) + BN1 + exp + maxpool(7) -> per-unit fc1 (83->100)
  + BN2 + relu -> per-unit fc2 (100->1) + BN3 + relu -> final linear (300->2).

Distribution: fully batch-sharded (16 b/core); per-core partial outputs are
8 partition-group partials of the final linear, summed on host.

v3 vs v2:
  - no DMA gating: w1 streams from t~0 in 3 unit-chunk pieces; fc1 of chunk
    ci is gated only on piece ci (tile slice deps)
  - w1 packed at per-unit stride 104 (not 128): LDWEIGHTS reads 128 cols
    overlapping the next unit's weights; garbage h rows 100..127 in psum are
    never evacuated.  5.3MB HBM instead of 6.5MB.
  - pexp stored [u, q, b] (q-major) so ONE dma_start_transpose per chunk
    ([128, 2048] -> [128q, 16b, 128u]) replaces 16 PE transposes + evacs
  - fc1/fc2 of chunk ci-1 interleaved into chunk ci's conv stream (PE dense)
  - fc2 batched: 8 units per matmul (N=8, diagonal-block psum [128, 304]);
    final linear = relu + per-class masked mul-reduce on DVE, host sums the
    8 partition groups
"""

import numpy as np
import ml_dtypes

B, N, L, K, C1 = 128, 300, 600, 19, 100
PS = 7
LP = 83            # pool windows
NCLS = 2
EPS = 1e-5

NCORES = 8
BLOC = B // NCORES            # 16 batch per core
NPAD = 304                    # units padded to 8*38
CK = 76                       # 4*19 contraction rows
WCONV_COLS = 384              # conv weight cols padded so every matmul is M=128
QP = 128                      # pexp q-cols: 83 pools + ones col 83 + pad;
                              # must equal the xbar tile width (128): the
                              # dma transpose maps out[q,b,u] = in[u, 128b+q]
US = 104                      # per-unit w1 column stride (16B aligned)

# packed-weight column offsets
W_CONV = 0
W_W2 = W_CONV + WCONV_COLS            # 384
W_MASK = W_W2 + NPAD                  # 688, 2 classes x 304
W_W1 = W_MASK + NCLS * NPAD           # 1296
W1_COLS = US * (NPAD - 1) + 128       # 31640 (last unit reads 128)
WTOT = W_W1 + W1_COLS

# conv matmul column splits: window-aligned 36/36/11 pool windows,
# each [128, 2, n] fp32 tile fits a single PSUM bank
CSPLIT = [(0, 252), (252, 252), (504, 78)]
QSPLIT = [(0, 36), (36, 36), (72, 11)]   # pool window ranges

# per-batch-PAIR pool mode: 'd'=DVE reduce_max direct from PSUM,
# 'a'=ACT copy to SBUF (w-major) + DVE bf16 pairwise-max tree
POOL_MODES = "ddaaaaaa"        # per pair (8 pairs); 4 direct batches, 12 copy

_CACHE = {}


def _build_bass():
    import concourse.bass as bass
    import concourse.bacc as bacc
    import concourse.mybir as mybir
    import concourse.tile as tile

    f32, bf16 = mybir.dt.float32, mybir.dt.bfloat16

    nc = bacc.Bacc("TRN2")
    xloc = nc.declare_dram_parameter("xloc", [4, BLOC, L], bf16, isOutput=False)
    wpack = nc.declare_dram_parameter("wpack", [128, WTOT], bf16, isOutput=False)
    c1p = nc.declare_dram_parameter("c1p", [128, 3], f32, isOutput=False)
    out_part = nc.declare_dram_parameter("out_part", [128, NCLS], f32, isOutput=True)

    n_copy = sum(2 for m in POOL_MODES if m != 'd')   # copy-path batches per chunk
    b_copy0 = 2 * POOL_MODES.index('a')

    with tile.TileContext(nc) as tc:
        with (
            tc.tile_pool(name="dram", bufs=1, space="DRAM") as dram_pool,
            tc.tile_pool(name="singles", bufs=1) as singles,
            tc.tile_pool(name="im2col", bufs=1) as im2col_pool,
            tc.tile_pool(name="praw", bufs=1) as praw_pool,
            tc.tile_pool(name="praws", bufs=1) as praws_pool,
            tc.tile_pool(name="gpst", bufs=1) as gpst_pool,
            tc.tile_pool(name="pexp", bufs=1) as pexp_pool,
        ):
            # ---------------- DMA triggers, small stuff first ----------
            wp_sb = singles.tile([128, WTOT], bf16)
            c1_sb = singles.tile([128, 3], f32)
            nc.scalar.dma_start(out=wp_sb[:, 0:W_W1], in_=wpack[:, 0:W_W1])
            nc.scalar.dma_start(out=c1_sb, in_=c1p[:, :])

            # im2col: [76, 16, 600] bf16; row (c*19+k), col (b, l) reads the
            # c-major flat x at 600b + l + k, one contiguous run per
            # partition (l+k <= 599 for all read cols so no b-row crossing).
            im2all = im2col_pool.tile([CK, BLOC, L], bf16, name="im2all")
            QL = 4 * L
            for bq in range(0, BLOC, 4):
                nrun = QL - (K - 1)
                src = bass.AP(
                    tensor=xloc,
                    offset=bq * L,
                    ap=[[BLOC * L, 4], [1, K], [1, nrun]],
                )
                nc.sync.dma_start(
                    out=im2all[:, bq:bq + 4, :].rearrange(
                        "p b l -> p (b l)")[:, 0:nrun],
                    in_=src)

            # w1 stream in 3 unit-chunk pieces (no gating; tile slice deps
            # let fc1 of chunk ci start when piece ci has landed)
            P0 = US * 128
            w1_pieces = [(0, P0), (P0, 2 * P0), (2 * P0, W1_COLS)]
            for (a, b) in w1_pieces:
                nc.gpsimd.dma_start(
                    out=wp_sb[0:LP + 1, W_W1 + a:W_W1 + b],
                    in_=wpack[0:LP + 1, W_W1 + a:W_W1 + b])

            wconv_sb = wp_sb[0:CK, W_CONV:W_CONV + WCONV_COLS]
            w2_sb = wp_sb[0:C1 + 1, W_W2:W_W2 + NPAD]
            mask_sb = wp_sb[0:128, W_MASK:W_MASK + NCLS * NPAD]

            praw = []       # pooled, BN1-normalized, pre-exp [128, 16b, 83q]
            praw_s = []     # raw conv rows staged for the DVE tree (w-major)
            pexp = []       # exp'd pooled features [128, 16b, 88q], q 83 = ones
            for ci in range(3):
                praw.append(praw_pool.tile([128, BLOC, LP], bf16, name=f"praw{ci}"))
                praw_s.append(praws_pool.tile([128, n_copy, PS * LP], bf16,
                                              name="praws", tag="praws"))
                p = pexp_pool.tile([128, BLOC, QP], bf16, name=f"pexp{ci}")
                nc.vector.memset(p[:, :, LP:QP], 1.0)
                pexp.append(p)

            # poolT2[q, b, u]: fc1 rhs for unit u = poolT2[0:84, :, u];
            # u padded to 3*128 for whole-block chunk transposes
            poolT2 = singles.tile([QP, BLOC, 384], bf16, name="poolT2")

            # h2B[(h), (u, b)] feeds fc2 directly (h on partitions)
            h2B = singles.tile([128, NPAD * BLOC], bf16, name="h2B")
            nc.vector.memset(h2B[96:128, :], 1.0)   # row 100 = fc2 bias ones

            w1_sb = wp_sb[0:LP + 1, W_W1:W_W1 + W1_COLS]

            with (
                tc.tile_pool(name="psA0", bufs=2, space="PSUM") as pool_a,
                tc.tile_pool(name="psA1", bufs=2, space="PSUM") as pool_b,
                tc.tile_pool(name="psA2", bufs=1, space="PSUM") as pool_c,
                tc.tile_pool(name="psB", bufs=2, space="PSUM") as psum_b,
                tc.tile_pool(name="psF", bufs=1, space="PSUM") as psumf_pool,
            ):
                psF = psumf_pool.tile([128, NPAD], f32, name="psF")

                def conv_pair(ci, bp, slot):
                    u0 = 128 * ci
                    lhsT = wconv_sb[:, u0:u0 + 128]
                    pss = [
                        pool_a.tile([128, 2, 252], f32, name="ps0", tag="ps0"),
                        pool_b.tile([128, 2, 252], f32, name="ps1", tag="ps1"),
                        pool_c.tile([128, 2, 78], f32, name="ps2", tag="ps2"),
                    ]
                    for pst, (l0, ncol) in zip(pss, CSPLIT):
                        nc.tensor.matmul(
                            out=pst[:, :, :],
                            lhsT=lhsT,
                            rhs=im2all[:, bp:bp + 2, l0:l0 + ncol],
                            start=True, stop=True,
                        )
                    mode = POOL_MODES[bp // 2]
                    if mode == 'd':
                        for pst, (q0, nq) in zip(pss, QSPLIT):
                            nc.vector.reduce_max(
                                out=praw[ci][:, bp:bp + 2, q0:q0 + nq],
                                in_=pst[:, :, 0:nq * PS].rearrange(
                                    "p s (q w) -> p s q w", w=PS),
                                axis=mybir.AxisListType.X,
                            )
                    else:
                        # copy in w-major order so the DVE tree below is
                        # contiguous (bf16 2x): col w*83+q <- psum 7q+w
                        view = praw_s[ci][:, slot:slot + 2, :].rearrange(
                            "p s (w q) -> p s w q", q=LP)
                        for pst, (q0, nq) in zip(pss, QSPLIT):
                            nc.scalar.copy(
                                out=view[:, :, :, q0:q0 + nq],
                                in_=pst[:, :, 0:nq * PS].rearrange(
                                    "p s (q w) -> p s w q", w=PS),
                            )

                def tree_exp_transpose(ci):
                    # DVE bf16 pairwise-max tree over the copy-path batches
                    if n_copy:
                        s = praw_s[ci]
                        w_of = lambda w: s[:, :, w * LP:(w + 1) * LP]
                        tA = gpst_pool.tile([128, n_copy, LP], bf16, name="tA", tag="tA")
                        tB = gpst_pool.tile([128, n_copy, LP], bf16, name="tB", tag="tB")
                        tC = gpst_pool.tile([128, n_copy, LP], bf16, name="tC", tag="tC")
                        tD = gpst_pool.tile([128, n_copy, LP], bf16, name="tD", tag="tD")
                        tE = gpst_pool.tile([128, n_copy, LP], bf16, name="tE", tag="tE")
                        nc.vector.tensor_max(out=tA, in0=w_of(0), in1=w_of(1))
                        nc.vector.tensor_max(out=tB, in0=w_of(2), in1=w_of(3))
                        nc.vector.tensor_max(out=tC, in0=w_of(4), in1=w_of(5))
                        nc.vector.tensor_max(out=tD, in0=tA, in1=tB)
                        nc.vector.tensor_max(out=tE, in0=tC, in1=w_of(6))
                        nc.vector.tensor_max(
                            out=praw[ci][:, b_copy0:b_copy0 + n_copy, :],
                            in0=tD, in1=tE)

                    # exp over the chunk's pooled features (pre-normalized)
                    nc.scalar.activation(
                        out=pexp[ci][:, :, 0:LP],
                        in_=praw[ci][:, :, :],
                        func=mybir.ActivationFunctionType.Exp,
                        bias=c1_sb[:, ci:ci + 1],
                    )

                    # one batched xbar transpose: [128u, (88q 16b)] ->
                    # [88q, 16b, 128u]  (8x11 xbar tiles of 16x128)
                    eng = nc.sync
                    eng.dma_start_transpose(
                        out=poolT2[:, :, 128 * ci:128 * ci + 128],
                        in_=pexp[ci].rearrange("p b q -> p (b q)"),
                    )

                def fc1_group(u0, nu, g):
                    # nu units: psum [128, nu*16] (<= one full bank)
                    psf = psum_b.tile([128, 32 * BLOC], f32, name="psf", tag="psf")
                    for k in range(nu):
                        u = u0 + k
                        nc.tensor.matmul(
                            out=psf[:, k * BLOC:(k + 1) * BLOC],
                            lhsT=w1_sb[:, US * u:US * u + 128],
                            rhs=poolT2[0:LP + 1, :, u],
                            start=True, stop=True,
                        )
                    ev_out = h2B[0:C1, u0 * BLOC:(u0 + nu) * BLOC]
                    ev_in = psf[0:C1, 0:nu * BLOC]
                    if g % 2 == 0:
                        nc.vector.tensor_scalar_max(out=ev_out, in0=ev_in,
                                                    scalar1=0.0)
                    else:
                        nc.scalar.activation(
                            out=ev_out, in_=ev_in,
                            func=mybir.ActivationFunctionType.Relu)

                def fc2_run(j0, nj):
                    # batched matmuls, each: 8 units, N=8, diagonal blocks
                    for j in range(j0, j0 + nj):
                        nc.tensor.matmul(
                            out=psF[:, 8 * j:8 * j + 8],
                            lhsT=h2B[0:C1 + 1, 128 * j:128 * (j + 1)],
                            rhs=w2_sb[:, 8 * j:8 * j + 8],
                            start=True, stop=True,
                        )

                def fc_slice(cj, p):
                    # 1/8 of chunk cj's fc work, interleaved into the next
                    # chunk's conv stream.  Chunks 0/1 have 128 units;
                    # chunk 2 has 48 (units 256..303).
                    if cj < 2:
                        if p < 4:
                            fc1_group(128 * cj + 32 * p, 32, p)
                        elif p < 6:
                            fc2_run(16 * cj + 8 * (p - 4), 8)
                    else:
                        if p == 0:
                            fc1_group(256, 32, 0)
                        elif p == 1:
                            fc1_group(288, 16, 1)
                        elif p == 2:
                            fc2_run(32, 6)

                for ci in range(3):
                    slot = 0
                    for bp in range(0, BLOC, 2):
                        conv_pair(ci, bp, slot)
                        if POOL_MODES[bp // 2] != 'd':
                            slot += 2
                        if ci > 0:
                            fc_slice(ci - 1, bp // 2)
                    tree_exp_transpose(ci)
                for p in range(3):
                    fc_slice(2, p)

                # ---------------- final linear ----------
                # psF[16n+b, u] valid iff n == u%8; relu then per-class
                # masked mul-reduce; host sums the 8 partition groups.
                h3relu = singles.tile([128, NPAD], bf16, name="h3relu")
                osum = singles.tile([128, NCLS], f32)
                nc.vector.tensor_scalar_max(out=h3relu, in0=psF, scalar1=0.0)
                prod = singles.tile([128, NPAD], f32)
                for cls in range(NCLS):
                    nc.vector.tensor_mul(
                        out=prod, in0=h3relu,
                        in1=mask_sb[:, cls * NPAD:(cls + 1) * NPAD])
                    nc.vector.reduce_sum(
                        out=osum[:, cls:cls + 1], in_=prod,
                        axis=mybir.AxisListType.X,
                    )
                nc.sync.dma_start(out=out_part[:, :], in_=osum)

    nc.finalize()
    return nc


def _host_prep(inputs):
    """Fold BN affines, pad units to 304, build per-core input maps."""
    x = np.asarray(inputs["x"], np.float32)
    conv_w = np.asarray(inputs["conv_w"], np.float32)
    conv_b = np.asarray(inputs["conv_b"], np.float32)
    g1, b1 = np.asarray(inputs["bn1_g"], np.float32), np.asarray(inputs["bn1_b"], np.float32)
    m1, v1 = np.asarray(inputs["bn1_m"], np.float32), np.asarray(inputs["bn1_v"], np.float32)
    fc1_w, fc1_b = np.asarray(inputs["fc1_w"], np.float32), np.asarray(inputs["fc1_b"], np.float32)
    g2, b2 = np.asarray(inputs["bn2_g"], np.float32), np.asarray(inputs["bn2_b"], np.float32)
    m2, v2 = np.asarray(inputs["bn2_m"], np.float32), np.asarray(inputs["bn2_v"], np.float32)
    fc2_w, fc2_b = np.asarray(inputs["fc2_w"], np.float32), np.asarray(inputs["fc2_b"], np.float32)
    g3, b3 = np.asarray(inputs["bn3_g"], np.float32), np.asarray(inputs["bn3_b"], np.float32)
    m3, v3 = np.asarray(inputs["bn3_m"], np.float32), np.asarray(inputs["bn3_v"], np.float32)
    final_w = np.asarray(inputs["final_w"], np.float32)
    final_b = np.asarray(inputs["final_b"], np.float32)

    a1 = g1 / np.sqrt(v1 + EPS)                      # [300] > 0
    c1 = a1 * (conv_b - m1) + b1                     # [300]
    a2 = g2 / np.sqrt(v2 + EPS)                      # [300,100]
    c2 = b2 - a2 * m2 + a2 * fc1_b                   # [300,100]
    a3 = g3 / np.sqrt(v3 + EPS)                      # [300]
    c3 = a3 * (fc2_b - m3) + b3                      # [300]

    bf = ml_dtypes.bfloat16

    # conv weights [76, 384]: a1 folded in; cols >= 300 are zero pad
    wconv = np.zeros((CK, WCONV_COLS), np.float32)
    wconv[0:76, :N] = (conv_w * a1[:, None, None]).transpose(1, 2, 0).reshape(76, N)
    c1t = np.zeros((128, 3), np.float32)
    for ci in range(3):
        u0 = 128 * ci
        seg = c1[u0:min(u0 + 128, N)]
        c1t[0:len(seg), ci] = seg

    # fc1: lhsT [84, >=100] per unit at stride US; rows 0..82 = a2*w1
    # (p-major), row 83 = c2
    w1aug = np.zeros((NPAD, LP + 1, C1), np.float32)
    w1aug[:N, :LP, :] = (fc1_w * a2[:, :, None]).transpose(0, 2, 1)
    w1aug[:N, LP, :] = c2
    w1c = np.zeros((LP + 1, W1_COLS), np.float32)
    for u in range(NPAD):
        w1c[:, US * u:US * u + C1] = w1aug[u]

    # fc2: rhs [101, 1] per unit; rows 0..99 = a3*w2, row 100 = c3
    w2aug = np.zeros((NPAD, C1 + 1), np.float32)
    w2aug[:N, :C1] = fc2_w * a3[:, None]
    w2aug[:N, C1] = c3

    # final-linear masks: mask[cls][16n+b, u] = final_w[cls, u] iff n == u%8
    masks = np.zeros((128, NCLS * NPAD), np.float32)
    for u in range(N):
        n_ = u % 8
        for cls in range(NCLS):
            masks[16 * n_:16 * n_ + BLOC, cls * NPAD + u] = final_w[cls, u]

    wp = np.zeros((128, WTOT), np.float32)
    wp[0:CK, W_CONV:W_CONV + WCONV_COLS] = wconv
    wp[0:C1 + 1, W_W2:W_W2 + NPAD] = w2aug.T
    wp[:, W_MASK:W_MASK + NCLS * NPAD] = masks
    wp[0:LP + 1, W_W1:W_W1 + W1_COLS] = w1c
    wp_bf = wp.astype(bf)

    in_maps = []
    for i in range(NCORES):
        in_maps.append({
            "xloc": np.ascontiguousarray(x[i * BLOC:(i + 1) * BLOC].transpose(1, 0, 2)).astype(bf),
            "wpack": wp_bf,
            "c1p": c1t,
        })
    return in_maps, final_b


def kernel(**inputs):
    from concourse.bass_utils import run_bass_kernel_spmd

    if "nc" not in _CACHE:
        _CACHE["nc"] = _build_bass()
    nc = _CACHE["nc"]

    in_maps, final_b = _host_prep(inputs)
    res = run_bass_kernel_spmd(nc, in_maps, core_ids=list(range(NCORES)))
    out = np.zeros((B, NCLS), np.float32)
    for i, r in enumerate(res.results):
        o = r["out_part"].reshape(8, BLOC, NCLS)   # [n, b, cls]
        out[i * BLOC:(i + 1) * BLOC] = o.sum(axis=0)
    out += final_b[None, :]
    return out
